# revision 51
# baseline (speedup 1.0000x reference)
"""Trainium2 kernel for nn_EulerBias: exact Riemann-solver bias field.

Structure:
  * Host (numpy, float32): the K-interface Newton solve (tiny: B x 63) ->
    wave speeds, then per-batch coefficient matrices for the device stage.
  * Device (8 NeuronCores, batch-parallel, 2 batches/core): for every query
    point q the bias over the 64 segment columns is

        out[q,k] = min(T1[q,k],0) + min(T2[q,k],0)

    where T1/T2 are affine in (u,it,1) = (x*it, 1/(t+eps), 1) with per-k
    coefficients -> one small-contraction (Kc=12) float32r matmul on TensorE
    (1 cyc/row vs fp32's 4) produces T1||T2 for 512 queries per instruction;
    ScalarE computes relu(-T2); VectorE fuses min(T1,0) - relu(-T2) in one
    op, writing bf16; DMA stores 512KB contiguous bf16 blocks (host upcasts
    to f32 - bias absmax ~2e5, bf16 keeps rel err ~2e-3, gate is 2e-2).

Masked columns (pieces_mask == 0) are encoded in the coefficients
(T1 = -1e9, T2 = +1e30) so no separate mask pass is needed. Assumes
pieces_mask >= 0 (it is a 0/1 mask; the harness fills ones).
"""

import numpy as np

GAMMA = np.float32(1.4)
EPS = np.float32(1e-6)
N_NEWTON = 20
B, K, NT, NX = 16, 64, 128, 256
NQ = NT * NX            # 32768 queries per batch
N_CORES = 8
B_PER_CORE = B // N_CORES
# device tiling: query q = sp*4096 + m*32 + h*16 + g*4 + j
#   m: psum partition (128), h: psum-tile half (2), g: matmul-in-half (4),
#   j: contraction group (4).  One sp-iteration = 4096 queries.
CHUNK = 128
GROUPS = 4
SP_Q = 4096                      # queries per sp iteration
SP_PER_BATCH = NQ // SP_Q        # 8
BIG = np.float32(1e30)
NEGBIG = np.float32(-1e9)

_COMPILED = {}


def _f32(x):
    return np.asarray(x, dtype=np.float32)


def _host_wave_speeds(xs, ks, ks_v, ks_p):
    """Mirror of reference.py's f32 Newton solve, in numpy float32."""
    gm1 = np.float32(GAMMA - 1.0)
    gp1 = np.float32(GAMMA + 1.0)
    exp_rare = np.float32(gm1 / (2.0 * GAMMA))

    def clip_lo(v, lo=EPS):
        return np.maximum(v, lo)

    rho_L, rho_R = ks[:, :-1], ks[:, 1:]
    u_L, u_R = ks_v[:, :-1], ks_v[:, 1:]
    p_L, p_R = ks_p[:, :-1], ks_p[:, 1:]

    def sound(rho, p):
        return np.sqrt(clip_lo(GAMMA * p / clip_lo(rho)))

    c_L, c_R = sound(rho_L, p_L), sound(rho_R, p_R)
    A_L = np.float32(2.0) / (gp1 * clip_lo(rho_L))
    A_R = np.float32(2.0) / (gp1 * clip_lo(rho_R))
    B_L = gm1 / gp1 * p_L
    B_R = gm1 / gp1 * p_R

    def wave_f_df(p, p_K, A_K, B_K, c_K):
        denom = clip_lo(p + B_K)
        sqrt_AoD = np.sqrt(clip_lo(A_K / denom))
        f_shock = (p - p_K) * sqrt_AoD
        df_shock = sqrt_AoD * (np.float32(1.0) - (p - p_K) / (np.float32(2.0) * denom))
        p_ratio = clip_lo(p / clip_lo(p_K))
        f_rare = np.float32(2.0) * c_K / gm1 * (p_ratio ** exp_rare - np.float32(1.0))
        df_rare = c_K / (GAMMA * clip_lo(p_K)) * p_ratio ** np.float32(-gp1 / (2.0 * GAMMA))
        is_shock = p > p_K
        return np.where(is_shock, f_shock, f_rare), np.where(is_shock, df_shock, df_rare)

    p0 = clip_lo(((c_L + c_R - gm1 / np.float32(2.0) * (u_R - u_L))
                  / (c_L / clip_lo(p_L) ** exp_rare + c_R / clip_lo(p_R) ** exp_rare))
                 ** np.float32(1.0 / exp_rare))
    p_star = p0
    for _ in range(N_NEWTON):
        f_L, df_L = wave_f_df(p_star, p_L, A_L, B_L, c_L)
        f_R, df_R = wave_f_df(p_star, p_R, A_R, B_R, c_R)
        residual = f_L + f_R + (u_R - u_L)
        jacobian = clip_lo(df_L + df_R)
        p_star = clip_lo(p_star - residual / jacobian)

    gp1_o_2g = np.float32(gp1 / (2.0 * GAMMA))
    sigma_1 = u_L - c_L * np.sqrt(clip_lo(np.float32(1.0) + gp1_o_2g * (p_star / clip_lo(p_L) - np.float32(1.0))))
    speed_left = np.where(p_star > p_L, sigma_1, u_L - c_L)
    sigma_3 = u_R + c_R * np.sqrt(clip_lo(np.float32(1.0) + gp1_o_2g * (p_star / clip_lo(p_R) - np.float32(1.0))))
    speed_right = np.where(p_star > p_R, sigma_3, u_R + c_R)
    return speed_left.astype(np.float32), speed_right.astype(np.float32)


def _host_coef(xs, mask, sl, sr):
    """Per-batch [12, 512] coefficient matrices (the matmul's moving operand).

    psum col n = 64*j + k       (j = contraction group) -> T1
    psum col n = 256 + 64*j + k                         -> T2
    contraction rows 3j+(0,1,2) multiply (u, it, 1) of group j.
    """
    xd = xs[:, 1:K]                      # (B, 63)
    m = mask.astype(np.float32)          # (B, 64)
    act = m != 0

    # T1 = -m*u + m*xd*it + m*sr   (k < 63);  col 63 -> +BIG;  masked -> -1e9
    Wu1 = np.zeros((B, K), np.float32)
    Wi1 = np.zeros((B, K), np.float32)
    Wc1 = np.zeros((B, K), np.float32)
    Wu1[:, :63] = -m[:, :63]
    Wi1[:, :63] = m[:, :63] * xd
    Wc1[:, :63] = m[:, :63] * sr
    Wc1[:, 63] = BIG
    Wu1[~act] = 0.0
    Wi1[~act] = 0.0
    Wc1[~act] = NEGBIG

    # T2 = m*u - m*xd[k-1]*it - m*sl[k-1] (k >= 1); col 0 or masked -> +BIG
    # (so min(T2,0) = -m*relu(sl[k-1] - xi[k-1]))
    Wu2 = np.zeros((B, K), np.float32)
    Wi2 = np.zeros((B, K), np.float32)
    Wc2 = np.zeros((B, K), np.float32)
    Wu2[:, 1:] = m[:, 1:]
    Wi2[:, 1:] = -m[:, 1:] * xd
    Wc2[:, 1:] = -m[:, 1:] * sl
    Wc2[:, 0] = BIG
    Wu2[~act] = 0.0
    Wi2[~act] = 0.0
    Wc2[~act] = BIG

    # double-bf16 contraction: xd*it = xd_hi*it_hi + xd_lo*it_hi + xd_hi*it_lo
    # (+O(2^-18) dropped), u = u_hi + u_lo.  bf16 x bf16 products are exact in
    # the f32 psum accumulate, so the matmul keeps ~f32 precision while
    # streaming 2-byte operands.  6 contraction rows per group j:
    #   [u_hi, u_lo, it_hi, it_hi, it_lo, 1]
    import ml_dtypes
    bf = ml_dtypes.bfloat16
    xd_hi = xd.astype(bf).astype(np.float32)
    xd_lo = (xd - xd_hi).astype(bf).astype(np.float32)
    rows1 = [Wu1, Wu1, None, None, None, Wc1]   # T1; it-rows filled per k
    rows2 = [Wu2, Wu2, None, None, None, Wc2]   # T2
    Wi1_hi = np.zeros((B, K), np.float32); Wi1_hi[:, :63] = m[:, :63] * xd_hi
    Wi1_lo = np.zeros((B, K), np.float32); Wi1_lo[:, :63] = m[:, :63] * xd_lo
    Wi2_hi = np.zeros((B, K), np.float32); Wi2_hi[:, 1:] = -m[:, 1:] * xd_hi
    Wi2_lo = np.zeros((B, K), np.float32); Wi2_lo[:, 1:] = -m[:, 1:] * xd_lo
    for W in (Wi1_hi, Wi1_lo, Wi2_hi, Wi2_lo):
        W[~act] = 0.0
    rows1[2], rows1[3], rows1[4] = Wi1_hi, Wi1_lo, Wi1_hi
    rows2[2], rows2[3], rows2[4] = Wi2_hi, Wi2_lo, Wi2_hi

    NR = 6
    coef = np.zeros((B, NR * GROUPS, 512), np.float32)
    for j in range(GROUPS):
        c1 = slice(64 * j, 64 * j + 64)
        c2 = slice(256 + 64 * j, 256 + 64 * j + 64)
        for r in range(NR):
            coef[:, NR * j + r, c1] = rows1[r]
            coef[:, NR * j + r, c2] = rows2[r]
    # replicate per matmul-slot g at partition rows 32g..32g+23 (rest of each
    # 32-row group zero-padded: a multi-partition-dim DMA dst scatters, so
    # loads must be plain [128, N] blocks) so four matmuls can run
    # concurrently in distinct 32-row groups of the PE array
    # (tile_position=(32g, 0))
    full = np.zeros((B, GROUPS, 32, 512), np.float32)
    full[:, :, 0:NR * GROUPS, :] = coef[:, None]
    return full.reshape(B, 128, 512).astype(bf)


def _host_qdata(t_coords, x_coords):
    """(B, SP, 4, 12, 256) stationary operands for query
    q = sp*4096 + m*32 + h*16 + g*4 + j: slot g's block [12, 256] has rows
    3j+(0,1,2) = (u, it, 1) and column 128*h + m.  On device slot g lands at
    partition rows 32g..32g+11 (tile_position row groups).

    The m-major query assignment makes each sp-iteration's store one
    contiguous 512KB bf16 HBM range (4KB per partition row)."""
    import ml_dtypes
    bf = ml_dtypes.bfloat16
    it = np.float32(1.0) / (t_coords.reshape(B, NQ) + EPS)
    u = x_coords.reshape(B, NQ) * it
    u_hi = u.astype(bf).astype(np.float32)
    u_lo = (u - u_hi).astype(np.float32)
    it_hi = it.astype(bf).astype(np.float32)
    it_lo = (it - it_hi).astype(np.float32)

    def lay(v):
        # (b, sp, m, h, g, j) -> [b, sp, g, j, (h, m)]
        v = v.reshape(B, SP_PER_BATCH, CHUNK, 2, GROUPS, GROUPS)
        return np.transpose(v, (0, 1, 4, 5, 3, 2)).reshape(
            B, SP_PER_BATCH, GROUPS, GROUPS, 2 * CHUNK)

    NR = 6
    qd = np.zeros((B, SP_PER_BATCH, GROUPS, 32, 2 * CHUNK), np.float32)
    for r, v in ((0, u_hi), (1, u_lo), (2, it_hi), (3, it_hi), (4, it_lo)):
        qd[:, :, :, r:NR * GROUPS:NR, :] = lay(v)
    qd[:, :, :, 5:NR * GROUPS:NR, :] = 1.0
    return qd.reshape(B, SP_PER_BATCH, 128, 2 * CHUNK).astype(bf)


def _build_nc(repeat=1):
    import concourse.bacc as bacc
    import concourse.mybir as mybir
    import concourse.tile as tile

    nc = bacc.Bacc(None, target_bir_lowering=False, debug=False)
    f32r = mybir.dt.float32r
    f32 = mybir.dt.float32
    bf16 = mybir.dt.bfloat16

    qd_d = nc.declare_dram_parameter(
        "qd", [B_PER_CORE, SP_PER_BATCH, 128, 2 * CHUNK], bf16, isOutput=False)
    cf_d = nc.declare_dram_parameter(
        "cf", [B_PER_CORE, 128, 512], bf16, isOutput=False)
    out_d = nc.declare_dram_parameter(
        "out", [B_PER_CORE, NQ, K], bf16, isOutput=True)

    with tile.TileContext(nc) as tc:
        with (
            tc.tile_pool(name="cf", bufs=1) as cfp,
            tc.tile_pool(name="qd", bufs=6) as qdp,
            tc.tile_pool(name="ps", bufs=4, space="PSUM") as psp,
            tc.tile_pool(name="p2", bufs=8) as p2p,
            tc.tile_pool(name="ot", bufs=6) as otp,
        ):
            cft = []
            for b in range(B_PER_CORE):
                c = cfp.tile([128, 512], bf16, tag=f"cf{b}")
                nc.sync.dma_start(c[:], cf_d[b])
                cft.append(c)
            n_iter = 0
            for _ in range(repeat):
                for b in range(B_PER_CORE):
                    for sp in range(SP_PER_BATCH):
                        qdt = qdp.tile([128, 2 * CHUNK], bf16)
                        # first load on the (empty) ACT HWDGE ring, parallel
                        # with cf on the SP ring: first matmul ~2us earlier
                        eng = nc.scalar if n_iter == 0 else nc.gpsimd
                        eng.dma_start(qdt[:], qd_d[b, sp])
                        n_iter += 1
                        ot = otp.tile([128, 2, GROUPS, 256], bf16)
                        # 2-bank psum tiles, 4 in rotation: the psum-reuse
                        # dependency cycle DVE(t) -> MM(t+4) -> ACT -> DVE
                        # amortizes over 4 tile-slots instead of 2
                        for h in range(2):
                            for gp in range(2):
                                ps = psp.tile([128, 2, 512], f32, name="ps")
                                if n_iter == 1 and h == 0 and gp == 0:
                                    # PE clock-gate warmup: garbage matmul off
                                    # the cf tile as soon as it lands, into a
                                    # bank the first real matmul overwrites -
                                    # starts the HAM ramp during the qd load
                                    nc.tensor.matmul(
                                        ps[:, 0, :], cft[b][0:24, 0:128],
                                        cft[b][0:24, :],
                                        start=True, stop=True)
                                for g2 in range(2):
                                    # four MMs per h (this pair + the gp=1
                                    # pair) sit in distinct 32-row PE groups
                                    # via tile_position -> they stream
                                    # concurrently on 4 XBUSes; without this
                                    # the half-clocked PE (50% duty never
                                    # releases the HAM throttle) serializes
                                    # 128 x 427ns = 55us of matmuls
                                    g = 2 * gp + g2
                                    nc.tensor.matmul(
                                        ps[:, g2, :],
                                        qdt[32 * g:32 * g + 24,
                                            128 * h:128 * (h + 1)],
                                        cft[b][32 * g:32 * g + 24, :],
                                        start=True, stop=True,
                                        tile_position=(32 * g, 0),
                                    )
                                p2 = p2p.tile([128, 2, 256], bf16)
                                nc.scalar.activation(
                                    p2[:], ps[:, :, 256:512],
                                    mybir.ActivationFunctionType.Relu, scale=-1.0)
                                nc.vector.scalar_tensor_tensor(
                                    out=ot[:, h, 2 * gp:2 * gp + 2],
                                    in0=ps[:, :, 0:256], scalar=0.0, in1=p2[:],
                                    op0=mybir.AluOpType.min,
                                    op1=mybir.AluOpType.subtract)
                        q0 = sp * SP_Q
                        dst = out_d[b, q0:q0 + SP_Q, :].rearrange(
                            "(m c) k -> m (c k)", c=32)
                        src = ot[:].rearrange("m h g x -> m (h g x)")
                        n_total = repeat * B_PER_CORE * SP_PER_BATCH
                        if n_iter == n_total:
                            # final store split across both HWDGE rings (all
                            # activations are done by then): halves the
                            # end-of-kernel drain
                            nc.scalar.dma_start(dst[:, 0:1024], src[:, 0:1024])
                            nc.sync.dma_start(dst[:, 1024:2048], src[:, 1024:2048])
                        elif n_iter % 4 == 0:
                            # every 4th store via SWDGE: keeps the SP ring
                            # (16 x 2.6us would be exactly DVE-co-critical)
                            # comfortably under the DVE period
                            nc.gpsimd.dma_start(dst, src)
                        else:
                            nc.sync.dma_start(dst, src)
    nc.compile()
    return nc


def _get_compiled(repeat=1):
    if repeat not in _COMPILED:
        _COMPILED[repeat] = _build_nc(repeat)
    return _COMPILED[repeat]


def _prep_inputs(inputs):
    xs = _f32(inputs["xs"])
    ks = _f32(inputs["ks"])
    ks_v = _f32(inputs["ks_v"])
    ks_p = _f32(inputs["ks_p"])
    mask = _f32(inputs["pieces_mask"])
    t_coords = _f32(inputs["t_coords"])
    x_coords = _f32(inputs["x_coords"])

    sl, sr = _host_wave_speeds(xs, ks, ks_v, ks_p)
    coef = _host_coef(xs, mask, sl, sr)
    qd = _host_qdata(t_coords, x_coords)
    return [
        {
            "qd": np.ascontiguousarray(qd[c * B_PER_CORE:(c + 1) * B_PER_CORE]),
            "cf": np.ascontiguousarray(coef[c * B_PER_CORE:(c + 1) * B_PER_CORE]),
        }
        for c in range(N_CORES)
    ]


def run(inputs, trace=False):
    from concourse.bass_utils import run_bass_kernel_spmd

    in_maps = _prep_inputs(inputs)
    nc = _get_compiled()
    res = None
    for attempt in range(3):
        try:
            res = run_bass_kernel_spmd(
                nc, in_maps, core_ids=list(range(N_CORES)), trace=trace)
            break
        except Exception:
            if attempt == 2:
                raise
            import time as _time
            _time.sleep(2.0)
    out = np.empty((B, NT, NX, K), np.float32)
    for c in range(N_CORES):
        out[c * B_PER_CORE:(c + 1) * B_PER_CORE] = (
            res.results[c]["out"].astype(np.float32).reshape(B_PER_CORE, NT, NX, K))
    return out, res


def kernel(**inputs):
    out, _ = run(inputs, trace=False)
    return out


# revision 52
# speedup vs baseline: 1.6375x; 1.6375x over previous
"""Trainium2 kernel for nn_EulerBias: exact Riemann-solver bias field.

Structure:
  * Host (numpy, float32): the K-interface Newton solve (tiny: B x 63) ->
    wave speeds, then per-batch coefficient matrices for the device stage.
  * Device (8 NeuronCores, batch-parallel, 2 batches/core): for every query
    point q the bias over the 64 segment columns is

        out[q,k] = min(T1[q,k],0) + min(T2[q,k],0)

    where T1/T2 are affine in (u,it,1) = (x*it, 1/(t+eps), 1) with per-k
    coefficients -> small-contraction bf16 matmuls on TensorE using a
    double-bf16 split (u_hi+u_lo, it_hi+it_lo; bf16 products are exact in
    the f32 psum accumulate -> ~f32 precision, Kc=24 rows/slot).  Four
    matmuls run concurrently in distinct 32-row PE groups via
    tile_position=(32g,0) - without this the half-clocked PE (~50% duty
    never releases the HAM throttle; f32r also streams at only ~1 row per
    0.83ns) serializes 128 x 427ns = 55us of matmuls, which ablations
    showed was the real end-to-end floor.  (f32r + tile_position hard-hangs
    the device - NRT_EXEC_UNIT_UNRECOVERABLE - hence bf16.)
    ScalarE computes relu(-T2); VectorE fuses min(T1,0) - relu(-T2) in one
    op, writing bf16; DMA stores 512KB contiguous bf16 blocks (host upcasts
    to f32 - bias absmax ~2e5, bf16 keeps rel err ~2e-3, gate is 2e-2).

Masked columns (pieces_mask == 0) are encoded in the coefficients
(T1 = -1e9, T2 = +1e30) so no separate mask pass is needed. Assumes
pieces_mask >= 0 (it is a 0/1 mask; the harness fills ones).
"""

import numpy as np

GAMMA = np.float32(1.4)
EPS = np.float32(1e-6)
N_NEWTON = 20
B, K, NT, NX = 16, 64, 128, 256
NQ = NT * NX            # 32768 queries per batch
N_CORES = 8
B_PER_CORE = B // N_CORES
# device tiling: query q = sp*4096 + m*32 + h*16 + g*4 + j
#   m: psum partition (128), h: psum-tile half (2), g: matmul-in-half (4),
#   j: contraction group (4).  One sp-iteration = 4096 queries.
CHUNK = 128
GROUPS = 4
SP_Q = 4096                      # queries per sp iteration
SP_PER_BATCH = NQ // SP_Q        # 8
BIG = np.float32(1e30)
NEGBIG = np.float32(-1e9)

_COMPILED = {}


def _f32(x):
    return np.asarray(x, dtype=np.float32)


def _host_wave_speeds(xs, ks, ks_v, ks_p):
    """Mirror of reference.py's f32 Newton solve, in numpy float32."""
    gm1 = np.float32(GAMMA - 1.0)
    gp1 = np.float32(GAMMA + 1.0)
    exp_rare = np.float32(gm1 / (2.0 * GAMMA))

    def clip_lo(v, lo=EPS):
        return np.maximum(v, lo)

    rho_L, rho_R = ks[:, :-1], ks[:, 1:]
    u_L, u_R = ks_v[:, :-1], ks_v[:, 1:]
    p_L, p_R = ks_p[:, :-1], ks_p[:, 1:]

    def sound(rho, p):
        return np.sqrt(clip_lo(GAMMA * p / clip_lo(rho)))

    c_L, c_R = sound(rho_L, p_L), sound(rho_R, p_R)
    A_L = np.float32(2.0) / (gp1 * clip_lo(rho_L))
    A_R = np.float32(2.0) / (gp1 * clip_lo(rho_R))
    B_L = gm1 / gp1 * p_L
    B_R = gm1 / gp1 * p_R

    def wave_f_df(p, p_K, A_K, B_K, c_K):
        denom = clip_lo(p + B_K)
        sqrt_AoD = np.sqrt(clip_lo(A_K / denom))
        f_shock = (p - p_K) * sqrt_AoD
        df_shock = sqrt_AoD * (np.float32(1.0) - (p - p_K) / (np.float32(2.0) * denom))
        p_ratio = clip_lo(p / clip_lo(p_K))
        f_rare = np.float32(2.0) * c_K / gm1 * (p_ratio ** exp_rare - np.float32(1.0))
        df_rare = c_K / (GAMMA * clip_lo(p_K)) * p_ratio ** np.float32(-gp1 / (2.0 * GAMMA))
        is_shock = p > p_K
        return np.where(is_shock, f_shock, f_rare), np.where(is_shock, df_shock, df_rare)

    p0 = clip_lo(((c_L + c_R - gm1 / np.float32(2.0) * (u_R - u_L))
                  / (c_L / clip_lo(p_L) ** exp_rare + c_R / clip_lo(p_R) ** exp_rare))
                 ** np.float32(1.0 / exp_rare))
    p_star = p0
    for _ in range(N_NEWTON):
        f_L, df_L = wave_f_df(p_star, p_L, A_L, B_L, c_L)
        f_R, df_R = wave_f_df(p_star, p_R, A_R, B_R, c_R)
        residual = f_L + f_R + (u_R - u_L)
        jacobian = clip_lo(df_L + df_R)
        p_star = clip_lo(p_star - residual / jacobian)

    gp1_o_2g = np.float32(gp1 / (2.0 * GAMMA))
    sigma_1 = u_L - c_L * np.sqrt(clip_lo(np.float32(1.0) + gp1_o_2g * (p_star / clip_lo(p_L) - np.float32(1.0))))
    speed_left = np.where(p_star > p_L, sigma_1, u_L - c_L)
    sigma_3 = u_R + c_R * np.sqrt(clip_lo(np.float32(1.0) + gp1_o_2g * (p_star / clip_lo(p_R) - np.float32(1.0))))
    speed_right = np.where(p_star > p_R, sigma_3, u_R + c_R)
    return speed_left.astype(np.float32), speed_right.astype(np.float32)


def _host_coef(xs, mask, sl, sr):
    """Per-batch [12, 512] coefficient matrices (the matmul's moving operand).

    psum col n = 64*j + k       (j = contraction group) -> T1
    psum col n = 256 + 64*j + k                         -> T2
    contraction rows 3j+(0,1,2) multiply (u, it, 1) of group j.
    """
    xd = xs[:, 1:K]                      # (B, 63)
    m = mask.astype(np.float32)          # (B, 64)
    act = m != 0

    # T1 = -m*u + m*xd*it + m*sr   (k < 63);  col 63 -> +BIG;  masked -> -1e9
    Wu1 = np.zeros((B, K), np.float32)
    Wi1 = np.zeros((B, K), np.float32)
    Wc1 = np.zeros((B, K), np.float32)
    Wu1[:, :63] = -m[:, :63]
    Wi1[:, :63] = m[:, :63] * xd
    Wc1[:, :63] = m[:, :63] * sr
    Wc1[:, 63] = BIG
    Wu1[~act] = 0.0
    Wi1[~act] = 0.0
    Wc1[~act] = NEGBIG

    # T2 = m*u - m*xd[k-1]*it - m*sl[k-1] (k >= 1); col 0 or masked -> +BIG
    # (so min(T2,0) = -m*relu(sl[k-1] - xi[k-1]))
    Wu2 = np.zeros((B, K), np.float32)
    Wi2 = np.zeros((B, K), np.float32)
    Wc2 = np.zeros((B, K), np.float32)
    Wu2[:, 1:] = m[:, 1:]
    Wi2[:, 1:] = -m[:, 1:] * xd
    Wc2[:, 1:] = -m[:, 1:] * sl
    Wc2[:, 0] = BIG
    Wu2[~act] = 0.0
    Wi2[~act] = 0.0
    Wc2[~act] = BIG

    # double-bf16 contraction: xd*it = xd_hi*it_hi + xd_lo*it_hi + xd_hi*it_lo
    # (+O(2^-18) dropped), u = u_hi + u_lo.  bf16 x bf16 products are exact in
    # the f32 psum accumulate, so the matmul keeps ~f32 precision while
    # streaming 2-byte operands.  6 contraction rows per group j:
    #   [u_hi, u_lo, it_hi, it_hi, it_lo, 1]
    import ml_dtypes
    bf = ml_dtypes.bfloat16
    xd_hi = xd.astype(bf).astype(np.float32)
    xd_lo = (xd - xd_hi).astype(bf).astype(np.float32)
    rows1 = [Wu1, Wu1, None, None, None, Wc1]   # T1; it-rows filled per k
    rows2 = [Wu2, Wu2, None, None, None, Wc2]   # T2
    Wi1_hi = np.zeros((B, K), np.float32); Wi1_hi[:, :63] = m[:, :63] * xd_hi
    Wi1_lo = np.zeros((B, K), np.float32); Wi1_lo[:, :63] = m[:, :63] * xd_lo
    Wi2_hi = np.zeros((B, K), np.float32); Wi2_hi[:, 1:] = -m[:, 1:] * xd_hi
    Wi2_lo = np.zeros((B, K), np.float32); Wi2_lo[:, 1:] = -m[:, 1:] * xd_lo
    for W in (Wi1_hi, Wi1_lo, Wi2_hi, Wi2_lo):
        W[~act] = 0.0
    rows1[2], rows1[3], rows1[4] = Wi1_hi, Wi1_lo, Wi1_hi
    rows2[2], rows2[3], rows2[4] = Wi2_hi, Wi2_lo, Wi2_hi

    NR = 6
    coef = np.zeros((B, NR * GROUPS, 512), np.float32)
    for j in range(GROUPS):
        c1 = slice(64 * j, 64 * j + 64)
        c2 = slice(256 + 64 * j, 256 + 64 * j + 64)
        for r in range(NR):
            coef[:, NR * j + r, c1] = rows1[r]
            coef[:, NR * j + r, c2] = rows2[r]
    # replicate per matmul-slot g at partition rows 32g..32g+23 (rest of each
    # 32-row group zero-padded: a multi-partition-dim DMA dst scatters, so
    # loads must be plain [128, N] blocks) so four matmuls can run
    # concurrently in distinct 32-row groups of the PE array
    # (tile_position=(32g, 0))
    full = np.zeros((B, GROUPS, 32, 512), np.float32)
    full[:, :, 0:NR * GROUPS, :] = coef[:, None]
    return full.reshape(B, 128, 512).astype(bf)


def _host_qdata(t_coords, x_coords):
    """(B, SP, 4, 12, 256) stationary operands for query
    q = sp*4096 + m*32 + h*16 + g*4 + j: slot g's block [12, 256] has rows
    3j+(0,1,2) = (u, it, 1) and column 128*h + m.  On device slot g lands at
    partition rows 32g..32g+11 (tile_position row groups).

    The m-major query assignment makes each sp-iteration's store one
    contiguous 512KB bf16 HBM range (4KB per partition row)."""
    import ml_dtypes
    bf = ml_dtypes.bfloat16
    it = np.float32(1.0) / (t_coords.reshape(B, NQ) + EPS)
    u = x_coords.reshape(B, NQ) * it
    u_hi = u.astype(bf).astype(np.float32)
    u_lo = (u - u_hi).astype(np.float32)
    it_hi = it.astype(bf).astype(np.float32)
    it_lo = (it - it_hi).astype(np.float32)

    def lay(v):
        # (b, sp, m, h, g, j) -> [b, sp, g, j, (h, m)]
        v = v.reshape(B, SP_PER_BATCH, CHUNK, 2, GROUPS, GROUPS)
        return np.transpose(v, (0, 1, 4, 5, 3, 2)).reshape(
            B, SP_PER_BATCH, GROUPS, GROUPS, 2 * CHUNK)

    NR = 6
    qd = np.zeros((B, SP_PER_BATCH, GROUPS, 32, 2 * CHUNK), np.float32)
    for r, v in ((0, u_hi), (1, u_lo), (2, it_hi), (3, it_hi), (4, it_lo)):
        qd[:, :, :, r:NR * GROUPS:NR, :] = lay(v)
    qd[:, :, :, 5:NR * GROUPS:NR, :] = 1.0
    return qd.reshape(B, SP_PER_BATCH, 128, 2 * CHUNK).astype(bf)


def _build_nc(repeat=1):
    import concourse.bacc as bacc
    import concourse.mybir as mybir
    import concourse.tile as tile

    nc = bacc.Bacc(None, target_bir_lowering=False, debug=False)
    f32r = mybir.dt.float32r
    f32 = mybir.dt.float32
    bf16 = mybir.dt.bfloat16

    qd_d = nc.declare_dram_parameter(
        "qd", [B_PER_CORE, SP_PER_BATCH, 128, 2 * CHUNK], bf16, isOutput=False)
    cf_d = nc.declare_dram_parameter(
        "cf", [B_PER_CORE, 128, 512], bf16, isOutput=False)
    out_d = nc.declare_dram_parameter(
        "out", [B_PER_CORE, NQ, K], bf16, isOutput=True)

    with tile.TileContext(nc) as tc:
        with (
            tc.tile_pool(name="cf", bufs=1) as cfp,
            tc.tile_pool(name="qd", bufs=6) as qdp,
            tc.tile_pool(name="ps", bufs=4, space="PSUM") as psp,
            tc.tile_pool(name="p2", bufs=8) as p2p,
            tc.tile_pool(name="ot", bufs=6) as otp,
        ):
            cft = []
            for b in range(B_PER_CORE):
                c = cfp.tile([128, 512], bf16, tag=f"cf{b}")
                nc.sync.dma_start(c[:], cf_d[b])
                cft.append(c)
            n_iter = 0
            for _ in range(repeat):
                for b in range(B_PER_CORE):
                    for sp in range(SP_PER_BATCH):
                        qdt = qdp.tile([128, 2 * CHUNK], bf16)
                        # first load on the (empty) ACT HWDGE ring, parallel
                        # with cf on the SP ring: first matmul ~2us earlier
                        eng = nc.scalar if n_iter == 0 else nc.gpsimd
                        eng.dma_start(qdt[:], qd_d[b, sp])
                        n_iter += 1
                        ot = otp.tile([128, 2, GROUPS, 256], bf16)
                        # 2-bank psum tiles, 4 in rotation: the psum-reuse
                        # dependency cycle DVE(t) -> MM(t+4) -> ACT -> DVE
                        # amortizes over 4 tile-slots instead of 2
                        for h in range(2):
                            for gp in range(2):
                                ps = psp.tile([128, 2, 512], f32, name="ps")
                                if n_iter == 1 and h == 0 and gp == 0:
                                    # PE clock-gate warmup: garbage matmul off
                                    # the cf tile as soon as it lands, into a
                                    # bank the first real matmul overwrites -
                                    # starts the HAM ramp during the qd load
                                    nc.tensor.matmul(
                                        ps[:, 0, :], cft[b][0:24, 0:128],
                                        cft[b][0:24, :],
                                        start=True, stop=True)
                                for g2 in range(2):
                                    # four MMs per h (this pair + the gp=1
                                    # pair) sit in distinct 32-row PE groups
                                    # via tile_position -> they stream
                                    # concurrently on 4 XBUSes; without this
                                    # the half-clocked PE (50% duty never
                                    # releases the HAM throttle) serializes
                                    # 128 x 427ns = 55us of matmuls
                                    g = 2 * gp + g2
                                    nc.tensor.matmul(
                                        ps[:, g2, :],
                                        qdt[32 * g:32 * g + 24,
                                            128 * h:128 * (h + 1)],
                                        cft[b][32 * g:32 * g + 24, :],
                                        start=True, stop=True,
                                        tile_position=(32 * g, 0),
                                    )
                                p2 = p2p.tile([128, 2, 256], bf16)
                                nc.scalar.activation(
                                    p2[:], ps[:, :, 256:512],
                                    mybir.ActivationFunctionType.Relu, scale=-1.0)
                                nc.vector.scalar_tensor_tensor(
                                    out=ot[:, h, 2 * gp:2 * gp + 2],
                                    in0=ps[:, :, 0:256], scalar=0.0, in1=p2[:],
                                    op0=mybir.AluOpType.min,
                                    op1=mybir.AluOpType.subtract)
                        q0 = sp * SP_Q
                        dst = out_d[b, q0:q0 + SP_Q, :].rearrange(
                            "(m c) k -> m (c k)", c=32)
                        src = ot[:].rearrange("m h g x -> m (h g x)")
                        n_total = repeat * B_PER_CORE * SP_PER_BATCH
                        if n_iter == n_total:
                            # final store split across both HWDGE rings (all
                            # activations are done by then): halves the
                            # end-of-kernel drain
                            nc.scalar.dma_start(dst[:, 0:1024], src[:, 0:1024])
                            nc.sync.dma_start(dst[:, 1024:2048], src[:, 1024:2048])
                        elif n_iter % 4 == 0:
                            # every 4th store via SWDGE: keeps the SP ring
                            # (16 x 2.6us would be exactly DVE-co-critical)
                            # comfortably under the DVE period
                            nc.gpsimd.dma_start(dst, src)
                        else:
                            nc.sync.dma_start(dst, src)
    nc.compile()
    return nc


def _get_compiled(repeat=1):
    if repeat not in _COMPILED:
        _COMPILED[repeat] = _build_nc(repeat)
    return _COMPILED[repeat]


def _prep_inputs(inputs):
    xs = _f32(inputs["xs"])
    ks = _f32(inputs["ks"])
    ks_v = _f32(inputs["ks_v"])
    ks_p = _f32(inputs["ks_p"])
    mask = _f32(inputs["pieces_mask"])
    t_coords = _f32(inputs["t_coords"])
    x_coords = _f32(inputs["x_coords"])

    sl, sr = _host_wave_speeds(xs, ks, ks_v, ks_p)
    coef = _host_coef(xs, mask, sl, sr)
    qd = _host_qdata(t_coords, x_coords)
    return [
        {
            "qd": np.ascontiguousarray(qd[c * B_PER_CORE:(c + 1) * B_PER_CORE]),
            "cf": np.ascontiguousarray(coef[c * B_PER_CORE:(c + 1) * B_PER_CORE]),
        }
        for c in range(N_CORES)
    ]


def run(inputs, trace=False):
    from concourse.bass_utils import run_bass_kernel_spmd

    in_maps = _prep_inputs(inputs)
    nc = _get_compiled()
    res = None
    for attempt in range(3):
        try:
            res = run_bass_kernel_spmd(
                nc, in_maps, core_ids=list(range(N_CORES)), trace=trace)
            break
        except Exception:
            if attempt == 2:
                raise
            import time as _time
            _time.sleep(2.0)
    out = np.empty((B, NT, NX, K), np.float32)
    for c in range(N_CORES):
        out[c * B_PER_CORE:(c + 1) * B_PER_CORE] = (
            res.results[c]["out"].astype(np.float32).reshape(B_PER_CORE, NT, NX, K))
    return out, res


def kernel(**inputs):
    out, _ = run(inputs, trace=False)
    return out
